# revision 9
# baseline (speedup 1.0000x reference)
"""GeneralSampleEdgeConv Trainium2 kernel, 8-core SPMD.

out = segment_sum(mask * (node_feature[src] ++ edge_feature) @ W_msg, dst)

v8 + dictionary-coded edge features: the int8-quantized edge-feature stream
is built from repeating 8-byte blocks (~8% unique). Host dedupes the blocks,
ships a shared dictionary (1/8 shard per core, AllGathered on device) plus
24-bit block indices per edge; the device reconstructs each chunk's ef tile
with 12 indirect-DMA block gathers. Lossless vs v8 (same int8 values).
"""
import math
import numpy as np

import jax

try:
    jax.config.update("jax_compilation_cache_dir", "/tmp/jax_comp_cache")
    jax.config.update("jax_persistent_cache_min_entry_size_bytes", -1)
    jax.config.update("jax_persistent_cache_min_compile_time_secs", 0)
except Exception:
    pass  # cache is an optimization only; never block execution

import concourse.tile as tile
from concourse import bass, bacc, mybir

F16 = mybir.dt.float16
F32 = mybir.dt.float32
I32 = mybir.dt.int32
I16 = mybir.dt.int16
I8 = mybir.dt.int8
U8 = mybir.dt.uint8

N, E, D = 50000, 800000, 96
PT = 128                      # nodes per tile
NT = math.ceil(N / PT)        # 391
NCORES = 8
SLOTS = math.ceil(NT / NCORES)  # 49 tile-slots per core
NTP = SLOTS * NCORES            # 392 padded tile count
NSH = N // NCORES               # 6250 node rows per core shard
NFROWS = 100                    # partitions carrying the nf shard segment
NFW = NSH * D // NFROWS         # 6000 bytes per carrying row
BS = 8                          # dict block size (int8 elements)
BPE = D // BS                   # 12 blocks per edge


def _offsets(CT, DSHW):
    ix_w = CT * BPE * 5 // 2     # 20-bit block indices, 2 per 5 bytes
    mt_w = CT * 4                # packed meta int32 bytes per row
    w_w = 384                    # W f16 bytes per row (192 cols)
    bpp = ix_w + mt_w + w_w + NFW + DSHW
    return ix_w, mt_w, w_w, bpp


def _build(cc_counts, U):
    """cc_counts[s] = chunks for tile-slot s; U = padded dict rows (8*128*k)."""
    CT = int(sum(cc_counts))
    DSH = U // NCORES            # dict rows per core shard (multiple of 128)
    DSHW = DSH // 128 * BS       # dict bytes per blob row
    ix_w, mt_w, w_w, bpp = _offsets(CT, DSHW)
    MOFF = ix_w
    WOFF = ix_w + mt_w
    NOFF = WOFF + w_w
    DOFF = NOFF + NFW

    nc = bacc.Bacc("TRN2", num_devices=NCORES)
    blob = nc.dram_tensor("blob", [128, bpp], I8, kind="ExternalInput")
    outm = nc.dram_tensor("outm", [128, SLOTS * (D + 4)], I8, kind="ExternalOutput")

    with tile.TileContext(nc) as tc:
        with (
            tc.tile_pool(name="const", bufs=1) as constp,
            tc.tile_pool(name="meta", bufs=1) as metap,
            tc.tile_pool(name="xg8", bufs=4) as xg8p,
            tc.tile_pool(name="xg", bufs=4) as xgp,
            tc.tile_pool(name="xf8", bufs=4) as xf8p,
            tc.tile_pool(name="xf", bufs=4) as xfp,
            tc.tile_pool(name="sb", bufs=3) as sb,
            tc.tile_pool(name="eplg", bufs=2) as ep,
            tc.tile_pool(name="psa", bufs=2, space="PSUM") as psa,
            tc.tile_pool(name="psb", bufs=2, space="PSUM") as psb,
            tc.tile_pool(name="pso", bufs=2, space="PSUM") as pso,
            tc.tile_pool(name="dram", bufs=1, space="DRAM") as dramp,
        ):
            csb8 = constp.tile([128, w_w], I8)
            nc.sync.dma_start(out=csb8[:], in_=blob[:, WOFF : WOFF + w_w])
            ccst = csb8.bitcast(F16)
            wt_sb = ccst[0:96, 0:96]
            wb_sb = ccst[0:96, 96:192]

            # iota row (each partition: 0..127), built on device
            iota_i = constp.tile([128, 128], I16)
            nc.gpsimd.iota(iota_i[:], pattern=[[1, 128]], base=0,
                           channel_multiplier=0)
            iota_t = constp.tile([128, 128], F16)
            nc.vector.tensor_copy(out=iota_t[:], in_=iota_i[:])
            iotaB = constp.tile([128, 16, 128], F16)
            for _g in range(16):
                nc.vector.tensor_copy(out=iotaB[:, _g, :], in_=iota_t[:])

            # meta: src index + dst_rel
            mt8 = metap.tile([128, mt_w], I8)
            nc.sync.dma_start(out=mt8[:], in_=blob[:, MOFF : MOFF + mt_w])
            mt_sb = mt8.bitcast(I32)
            si_sb = metap.tile([128, CT], I32)
            nc.vector.tensor_scalar(
                out=si_sb[:], in0=mt_sb[:], scalar1=8, scalar2=None,
                op0=mybir.AluOpType.logical_shift_right)
            dr32 = metap.tile([128, CT], I32)
            nc.vector.tensor_scalar(
                out=dr32[:], in0=mt_sb[:], scalar1=255, scalar2=None,
                op0=mybir.AluOpType.bitwise_and)
            dr_sb = metap.tile([128, CT], F16)
            nc.vector.tensor_copy(out=dr_sb[:], in_=dr32[:])

            # block indices: int24 -> int32 (zero-extended byte assembly);
            # temporaries live in a scratch pool that closes before the
            # main loop so their SBUF is reclaimed.
            nidx = CT * BPE
            nhalf = nidx // 2
            bi_sb = metap.tile([128, nidx], I32)
            with tc.tile_pool(name="ixtmp", bufs=1) as ixtmp:
                ixb = ixtmp.tile([128, ix_w], U8)
                nc.sync.dma_start(out=ixb[:], in_=blob[:, 0:ix_w].bitcast(U8))
                bb = []
                for k in range(5):
                    t = ixtmp.tile([128, nhalf], I32, name=f"bb{k}")
                    nc.vector.tensor_copy(out=t[:], in_=ixb[:, k::5])
                    bb.append(t)
                b0, b1, b2, b3, b4 = bb
                # even idx = b0 | b1<<8 | (b2 & 15)<<16
                ev = ixtmp.tile([128, nhalf], I32)
                nc.vector.tensor_scalar(
                    out=ev[:], in0=b2[:], scalar1=15, scalar2=None,
                    op0=mybir.AluOpType.bitwise_and)
                nc.vector.tensor_scalar(
                    out=ev[:], in0=ev[:], scalar1=16, scalar2=None,
                    op0=mybir.AluOpType.logical_shift_left)
                nc.vector.tensor_scalar(
                    out=b1[:], in0=b1[:], scalar1=8, scalar2=None,
                    op0=mybir.AluOpType.logical_shift_left)
                nc.vector.tensor_tensor(out=ev[:], in0=ev[:], in1=b1[:],
                                        op=mybir.AluOpType.bitwise_or)
                nc.vector.tensor_tensor(out=ev[:], in0=ev[:], in1=b0[:],
                                        op=mybir.AluOpType.bitwise_or)
                # odd idx = b2>>4 | b3<<4 | b4<<12
                od = ixtmp.tile([128, nhalf], I32)
                nc.vector.tensor_scalar(
                    out=od[:], in0=b2[:], scalar1=4, scalar2=None,
                    op0=mybir.AluOpType.logical_shift_right)
                nc.vector.tensor_scalar(
                    out=b3[:], in0=b3[:], scalar1=4, scalar2=None,
                    op0=mybir.AluOpType.logical_shift_left)
                nc.vector.tensor_scalar(
                    out=b4[:], in0=b4[:], scalar1=12, scalar2=None,
                    op0=mybir.AluOpType.logical_shift_left)
                nc.vector.tensor_tensor(out=od[:], in0=od[:], in1=b3[:],
                                        op=mybir.AluOpType.bitwise_or)
                nc.vector.tensor_tensor(out=od[:], in0=od[:], in1=b4[:],
                                        op=mybir.AluOpType.bitwise_or)
                nc.vector.tensor_copy(out=bi_sb[:, 0::2], in_=ev[:])
                nc.vector.tensor_copy(out=bi_sb[:, 1::2], in_=od[:])

            # AllGather node table shard -> full [N, D] int8 table
            nf_in = dramp.tile([NFROWS, NFW], I8)
            nf_all = dramp.tile([N, D], I8)
            nc.gpsimd.dma_start(out=nf_in[:], in_=blob[0:NFROWS, NOFF : NOFF + NFW])
            nc.gpsimd.collective_compute(
                "AllGather", mybir.AluOpType.bypass,
                replica_groups=[list(range(NCORES))],
                ins=[nf_in.opt()], outs=[nf_all.opt()],
            )
            # AllGather ef-block dictionary shard -> full [U, BS] int8 table
            dc_in = dramp.tile([128, DSHW], I8)
            dc_all = dramp.tile([U, BS], I8)
            nc.gpsimd.dma_start(out=dc_in[:], in_=blob[:, DOFF : DOFF + DSHW])
            nc.gpsimd.collective_compute(
                "AllGather", mybir.AluOpType.bypass,
                replica_groups=[list(range(NCORES))],
                ins=[dc_in.opt()], outs=[dc_all.opt()],
            )

            # grouped pre-build: per chunk, gather x_j rows (1) and ef blocks
            # (BPE); cast both to f16; build one-hots. Then per-chunk matmuls
            # accumulate transposed: pa[f,n] += Xg.T @ P ; pb[f,n] += EF.T @ P
            GS = 16
            pgroups = {}

            def group_of(c):
                g = c // GS
                if g not in pgroups:
                    gs = min(GS, CT - g * GS)
                    xgB = xg8p.tile([128, GS * D], I8, tag="xg8")
                    xfB = xf8p.tile([128, GS * D], I8, tag="xf8")
                    for j in range(gs):
                        cc = g * GS + j
                        nc.gpsimd.indirect_dma_start(
                            out=xgB[:, j * D : (j + 1) * D],
                            out_offset=None,
                            in_=nf_all[:],
                            in_offset=bass.IndirectOffsetOnAxis(
                                ap=si_sb[:, cc : cc + 1], axis=0),
                        )
                        for k in range(BPE):
                            nc.gpsimd.indirect_dma_start(
                                out=xfB[:, j * D + k * BS : j * D + (k + 1) * BS],
                                out_offset=None,
                                in_=dc_all[:],
                                in_offset=bass.IndirectOffsetOnAxis(
                                    ap=bi_sb[:, cc * BPE + k : cc * BPE + k + 1],
                                    axis=0),
                            )
                    xgB16 = xgp.tile([128, GS * D], F16, tag="xg")
                    nc.vector.tensor_copy(out=xgB16[:, : gs * D],
                                          in_=xgB[:, : gs * D])
                    xfB16 = xfp.tile([128, GS * D], F16, tag="xf")
                    nc.vector.tensor_copy(out=xfB16[:, : gs * D],
                                          in_=xfB[:, : gs * D])
                    PB = sb.tile([128, GS, 128], F16, tag="onehot")
                    nc.vector.tensor_tensor(
                        out=PB[:, :gs, :],
                        in0=dr_sb[:, g * GS : g * GS + gs].to_broadcast(
                            [128, gs, 128]),
                        in1=iotaB[:, :gs, :],
                        op=mybir.AluOpType.is_equal,
                    )
                    pgroups[g] = (xgB16, xfB16, PB)
                return pgroups[g], c - g * GS

            obig = constp.tile([128, SLOTS, D], F32)
            cur = 0
            for s in range(SLOTS):
                pa = psa.tile([D, 128], F32, tag="pa")
                pb = psb.tile([D, 128], F32, tag="pb")
                nch = int(cc_counts[s])
                for j in range(nch):
                    c = cur + j
                    (xgB16, xfB16, PB), gj = group_of(c)
                    nc.tensor.matmul(
                        out=pa[:], lhsT=xgB16[:, gj * D : (gj + 1) * D],
                        rhs=PB[:, gj, :],
                        start=(j == 0), stop=(j == nch - 1),
                    )
                    nc.tensor.matmul(
                        out=pb[:], lhsT=xfB16[:, gj * D : (gj + 1) * D],
                        rhs=PB[:, gj, :],
                        start=(j == 0), stop=(j == nch - 1),
                    )
                cur += nch

                aT = ep.tile([D, 128], F16, tag="aT")
                nc.vector.tensor_copy(out=aT[:], in_=pa[:])
                bT = ep.tile([D, 128], F16, tag="bT")
                nc.vector.tensor_copy(out=bT[:], in_=pb[:])
                ops = pso.tile([128, D], F32, tag="ops")
                nc.tensor.matmul(out=ops[:], lhsT=aT[:], rhs=wt_sb, start=True, stop=False)
                nc.tensor.matmul(out=ops[:], lhsT=bT[:], rhs=wb_sb, start=False, stop=True)
                nc.vector.tensor_copy(out=obig[:, s, :], in_=ops[:])

            # batched quantization epilogue over all slots
            am = constp.tile([128, SLOTS], F32)
            nc.vector.tensor_reduce(
                out=am[:], in_=obig[:, :, :], axis=mybir.AxisListType.X,
                op=mybir.AluOpType.max, apply_absolute_value=True)
            nc.vector.tensor_scalar_max(out=am[:], in0=am[:], scalar1=1e-12)
            rc = constp.tile([128, SLOTS], F32)
            nc.vector.reciprocal(out=rc[:], in_=am[:])
            nc.vector.tensor_scalar_mul(out=rc[:], in0=rc[:], scalar1=127.0)
            osb = constp.tile([128, SLOTS, D + 4], I8)
            nc.vector.tensor_tensor(
                out=osb[:, :, 0:D], in0=obig[:, :, :],
                in1=rc[:, :].to_broadcast([128, SLOTS, D]),
                op=mybir.AluOpType.mult)
            nc.vector.tensor_scalar_mul(
                out=osb[:, :, D : D + 4].bitcast(F32), in0=am[:, :],
                scalar1=1.0 / 127.0)
            nc.sync.dma_start(out=outm[:, :], in_=osb[:])
    nc.compile()
    return nc


def _prep(node_feature, edge_feature, edge_index, edge_mask):
    """Host shard: returns (cc_counts, U, blobs, scales, tiles_of_core)."""
    src = np.asarray(edge_index[0], dtype=np.int64)
    dst = np.asarray(edge_index[1], dtype=np.int64)
    keep = np.asarray(edge_mask, dtype=bool)
    src, dst = src[keep], dst[keep]
    ef32 = np.asarray(edge_feature, dtype=np.float32)[keep]
    ef_scale = float(np.abs(ef32).max()) / 127.0
    ef = np.clip(np.round(ef32 / ef_scale), -127, 127).astype(np.int8)
    nf32 = np.asarray(node_feature, dtype=np.float32)
    nf_scale = float(np.abs(nf32).max()) / 127.0
    nf8 = np.clip(np.round(nf32 / nf_scale), -127, 127).astype(np.int8)

    # dictionary-code the int8 ef stream as aligned 8-byte blocks
    blocks = ef.reshape(-1, BS)
    bv = blocks.view([('', np.int8)] * BS).ravel()
    uniq, inv = np.unique(bv, return_inverse=True)
    nuniq = len(uniq)
    U = ((nuniq + NCORES * 128 - 1) // (NCORES * 128)) * (NCORES * 128)
    dict_full = np.zeros((U, BS), np.int8)
    dict_full[:nuniq] = uniq.view(np.int8).reshape(nuniq, BS)
    eidx = inv.astype(np.int32).reshape(-1, BPE)   # [kept_E, BPE]

    tid = dst >> 7
    order = np.argsort(tid, kind="stable")
    src, dst, eidx, tid = src[order], dst[order], eidx[order], tid[order]
    cnt = np.bincount(tid, minlength=NTP)
    starts = np.concatenate([[0], np.cumsum(cnt)])

    # snake-deal tiles (desc count) to cores
    rank = np.argsort(-cnt, kind="stable")
    tiles_of_core = [[] for _ in range(NCORES)]
    for r, t in enumerate(rank):
        blk, pos = divmod(r, NCORES)
        c = pos if blk % 2 == 0 else NCORES - 1 - pos
        tiles_of_core[c].append(int(t))

    # per-slot chunk counts: max over cores
    cc_counts = np.ones(SLOTS, np.int64)
    for s in range(SLOTS):
        m = max(cnt[tiles_of_core[c][s]] for c in range(NCORES))
        cc_counts[s] = max(1, math.ceil(m / PT))
    CT = int(cc_counts.sum())
    DSH = U // NCORES
    DSHW = DSH // 128 * BS
    ix_w, mt_w, w_w, bpp = _offsets(CT, DSHW)
    MOFF = ix_w
    WOFF = ix_w + mt_w
    NOFF = WOFF + w_w
    DOFF = NOFF + NFW

    blobs = []
    for c in range(NCORES):
        ia = np.zeros((CT * PT, BPE), np.int32)
        mt = np.full(CT * PT, 255, np.int32)  # pad: src=0, dst_rel=255
        cur = 0
        for s in range(SLOTS):
            t = tiles_of_core[c][s]
            e0, e1 = starts[t], starts[t] + cnt[t]
            n = e1 - e0
            o = cur * PT
            ia[o : o + n] = eidx[e0:e1]
            mt[o : o + n] = (src[e0:e1] * 256 + (dst[e0:e1] - t * PT)).astype(np.int32)
            cur += int(cc_counts[s])
        blob = np.zeros((PT, bpp), np.int8)
        # idx region: partition-major [chunk, slot(p), BPE] -> rows p,
        # int24 little-endian
        iam = ia.reshape(CT, PT, BPE).transpose(1, 0, 2).reshape(PT, CT * BPE)
        ev, od = iam[:, 0::2], iam[:, 1::2]
        i20 = np.empty((PT, CT * BPE // 2, 5), np.uint8)
        i20[:, :, 0] = ev & 255
        i20[:, :, 1] = (ev >> 8) & 255
        i20[:, :, 2] = ((ev >> 16) & 15) | ((od & 15) << 4)
        i20[:, :, 3] = (od >> 4) & 255
        i20[:, :, 4] = (od >> 12) & 255
        blob[:, :ix_w] = i20.reshape(PT, ix_w).view(np.int8)
        blob[:, MOFF : MOFF + mt_w] = (
            np.ascontiguousarray(mt.reshape(CT, PT).T).view(np.int8))
        nfseg = np.ascontiguousarray(
            nf8[c * NSH:(c + 1) * NSH]).reshape(NFROWS, NFW)
        blob[0:NFROWS, NOFF : NOFF + NFW] = nfseg
        blob[:, DOFF : DOFF + DSHW] = (
            dict_full[c * DSH:(c + 1) * DSH].reshape(PT, DSHW))
        blobs.append(blob)
    return cc_counts, U, blobs, (ef_scale, nf_scale), tiles_of_core


def _in_maps(inputs, cc_counts, U, blobs, scales):
    ef_scale, nf_scale = scales
    CT = int(sum(cc_counts))
    DSHW = U // NCORES // 128 * BS
    ix_w, mt_w, w_w, bpp = _offsets(CT, DSHW)
    WOFF = ix_w + mt_w
    w32 = np.asarray(inputs["W_msg"], np.float32)
    consts = np.zeros((128, 192), np.float16)
    consts[:96, 0:96] = (w32[:96] * nf_scale).astype(np.float16)
    consts[:96, 96:192] = (w32[96:] * ef_scale).astype(np.float16)
    cbytes = consts.view(np.int8)
    maps = []
    for c in range(NCORES):
        b = blobs[c].copy()
        b[:, WOFF : WOFF + w_w] = cbytes
        maps.append({"blob": b})
    return maps


def make_in_maps(inputs, prep):
    return _in_maps(inputs, prep[0], prep[1], prep[2], prep[3])


def kernel(node_feature, edge_feature, edge_index, edge_mask, W_msg):
    from concourse.bass_utils import run_bass_kernel_spmd

    cc_counts, U, blobs, scales, tiles_of_core = _prep(
        node_feature, edge_feature, edge_index, edge_mask)
    nc = _build(cc_counts, U)
    in_maps = _in_maps({"W_msg": W_msg}, cc_counts, U, blobs, scales)

    res = run_bass_kernel_spmd(nc, in_maps, list(range(NCORES)))

    out_full = np.zeros((NTP * PT, D), np.float32)
    for c in range(NCORES):
        om = res.results[c]["outm"].reshape(128, SLOTS, D + 4)
        q = om[:, :, :D].astype(np.float32)          # [128, SLOTS, D]
        scl = np.ascontiguousarray(om[:, :, D:]).view(np.float32)[:, :, 0]
        oc = (q * scl[:, :, None]).transpose(1, 0, 2)  # [SLOTS, 128, D]
        for s in range(SLOTS):
            t = tiles_of_core[c][s]
            out_full[t * PT : (t + 1) * PT] = oc[s]
    return out_full[:N]


def build_from_prep(prep):
    return _build(prep[0], prep[1])


# revision 10
# speedup vs baseline: 1.4572x; 1.4572x over previous
"""GeneralSampleEdgeConv Trainium2 kernel, 8-core SPMD.

out = segment_sum(mask * (node_feature[src] ++ edge_feature) @ W_msg, dst)

v8 + dictionary-coded edge features: the int8-quantized edge-feature stream
is built from repeating 8-byte blocks (~8% unique). Host dedupes the blocks,
ships a shared dictionary (1/8 shard per core, AllGathered on device) plus
24-bit block indices per edge; the device reconstructs each chunk's ef tile
with 12 indirect-DMA block gathers. Lossless vs v8 (same int8 values).
"""
import math
import numpy as np

import jax

try:
    jax.config.update("jax_compilation_cache_dir", "/tmp/jax_comp_cache")
    jax.config.update("jax_persistent_cache_min_entry_size_bytes", -1)
    jax.config.update("jax_persistent_cache_min_compile_time_secs", 0)
except Exception:
    pass  # cache is an optimization only; never block execution

import concourse.tile as tile
from concourse import bass, bacc, mybir

F16 = mybir.dt.float16
F32 = mybir.dt.float32
I32 = mybir.dt.int32
I16 = mybir.dt.int16
I8 = mybir.dt.int8
U8 = mybir.dt.uint8

N, E, D = 50000, 800000, 96
PT = 128                      # nodes per tile
NT = math.ceil(N / PT)        # 391
NCORES = 8
SLOTS = math.ceil(NT / NCORES)  # 49 tile-slots per core
NTP = SLOTS * NCORES            # 392 padded tile count
NSH = N // NCORES               # 6250 node rows per core shard
NFROWS = 100                    # partitions carrying the nf shard segment
NFW = NSH * D // NFROWS         # 6000 bytes per carrying row
BS = 8                          # dict block size (int8 elements)
BPE = D // BS                   # 12 blocks per edge


def _offsets(CT, DSHW):
    ix_w = CT * BPE * 5 // 2     # 20-bit block indices, 2 per 5 bytes
    mt_w = CT * 3                # packed meta int24 bytes per row
    w_w = 384                    # W f16 bytes per row (192 cols)
    bpp = ix_w + mt_w + w_w + NFW + DSHW
    return ix_w, mt_w, w_w, bpp


def _build(cc_counts, U):
    """cc_counts[s] = chunks for tile-slot s; U = padded dict rows (8*128*k)."""
    CT = int(sum(cc_counts))
    DSH = U // NCORES            # dict rows per core shard (multiple of 128)
    DSHW = DSH // 128 * BS       # dict bytes per blob row
    ix_w, mt_w, w_w, bpp = _offsets(CT, DSHW)
    MOFF = ix_w
    WOFF = ix_w + mt_w
    NOFF = WOFF + w_w
    DOFF = NOFF + NFW

    nc = bacc.Bacc("TRN2", num_devices=NCORES)
    blob = nc.dram_tensor("blob", [128, bpp], I8, kind="ExternalInput")
    outm = nc.dram_tensor("outm", [128, SLOTS * (D + 4)], I8, kind="ExternalOutput")

    with tile.TileContext(nc) as tc:
        with (
            tc.tile_pool(name="const", bufs=1) as constp,
            tc.tile_pool(name="meta", bufs=1) as metap,
            tc.tile_pool(name="xg8", bufs=4) as xg8p,
            tc.tile_pool(name="xg", bufs=4) as xgp,
            tc.tile_pool(name="xf8", bufs=4) as xf8p,
            tc.tile_pool(name="xf", bufs=4) as xfp,
            tc.tile_pool(name="sb", bufs=3) as sb,
            tc.tile_pool(name="eplg", bufs=2) as ep,
            tc.tile_pool(name="psa", bufs=2, space="PSUM") as psa,
            tc.tile_pool(name="psb", bufs=2, space="PSUM") as psb,
            tc.tile_pool(name="pso", bufs=2, space="PSUM") as pso,
            tc.tile_pool(name="dram", bufs=1, space="DRAM") as dramp,
        ):
            csb8 = constp.tile([128, w_w], I8)
            nc.sync.dma_start(out=csb8[:], in_=blob[:, WOFF : WOFF + w_w])
            ccst = csb8.bitcast(F16)
            wt_sb = ccst[0:96, 0:96]
            wb_sb = ccst[0:96, 96:192]

            # iota row (each partition: 0..127), built on device
            iota_i = constp.tile([128, 128], I16)
            nc.gpsimd.iota(iota_i[:], pattern=[[1, 128]], base=0,
                           channel_multiplier=0)
            iota_t = constp.tile([128, 128], F16)
            nc.vector.tensor_copy(out=iota_t[:], in_=iota_i[:])
            iotaB = constp.tile([128, 16, 128], F16)
            for _g in range(16):
                nc.vector.tensor_copy(out=iotaB[:, _g, :], in_=iota_t[:])

            # meta: src index + dst_rel, shipped as int24
            si_sb = metap.tile([128, CT], I32)
            dr_sb = metap.tile([128, CT], F16)
            with tc.tile_pool(name="mttmp", bufs=1) as mttmp:
                mt8 = mttmp.tile([128, mt_w], U8)
                nc.sync.dma_start(out=mt8[:],
                                  in_=blob[:, MOFF : MOFF + mt_w].bitcast(U8))
                m0 = mttmp.tile([128, CT], I32)
                nc.vector.tensor_copy(out=m0[:], in_=mt8[:, 0::3])
                m1 = mttmp.tile([128, CT], I32)
                nc.vector.tensor_copy(out=m1[:], in_=mt8[:, 1::3])
                m2 = mttmp.tile([128, CT], I32)
                nc.vector.tensor_copy(out=m2[:], in_=mt8[:, 2::3])
                # dst_rel is the low byte; src = m1 | m2<<8
                nc.vector.tensor_copy(out=dr_sb[:], in_=m0[:])
                nc.vector.tensor_scalar(
                    out=m2[:], in0=m2[:], scalar1=8, scalar2=None,
                    op0=mybir.AluOpType.logical_shift_left)
                nc.vector.tensor_tensor(out=si_sb[:], in0=m1[:], in1=m2[:],
                                        op=mybir.AluOpType.bitwise_or)

            # block indices: int24 -> int32 (zero-extended byte assembly);
            # temporaries live in a scratch pool that closes before the
            # main loop so their SBUF is reclaimed.
            nidx = CT * BPE
            nhalf = nidx // 2
            bi_sb = metap.tile([128, nidx], I32)
            with tc.tile_pool(name="ixtmp", bufs=1) as ixtmp:
                ixb = ixtmp.tile([128, ix_w], U8)
                nc.sync.dma_start(out=ixb[:], in_=blob[:, 0:ix_w].bitcast(U8))
                bb = []
                for k in range(5):
                    t = ixtmp.tile([128, nhalf], I32, name=f"bb{k}")
                    nc.vector.tensor_copy(out=t[:], in_=ixb[:, k::5])
                    bb.append(t)
                b0, b1, b2, b3, b4 = bb
                # even idx = b0 | b1<<8 | (b2 & 15)<<16
                ev = ixtmp.tile([128, nhalf], I32)
                nc.vector.tensor_scalar(
                    out=ev[:], in0=b2[:], scalar1=15, scalar2=None,
                    op0=mybir.AluOpType.bitwise_and)
                nc.vector.tensor_scalar(
                    out=ev[:], in0=ev[:], scalar1=16, scalar2=None,
                    op0=mybir.AluOpType.logical_shift_left)
                nc.vector.tensor_scalar(
                    out=b1[:], in0=b1[:], scalar1=8, scalar2=None,
                    op0=mybir.AluOpType.logical_shift_left)
                nc.vector.tensor_tensor(out=ev[:], in0=ev[:], in1=b1[:],
                                        op=mybir.AluOpType.bitwise_or)
                nc.vector.tensor_tensor(out=ev[:], in0=ev[:], in1=b0[:],
                                        op=mybir.AluOpType.bitwise_or)
                # odd idx = b2>>4 | b3<<4 | b4<<12
                od = ixtmp.tile([128, nhalf], I32)
                nc.vector.tensor_scalar(
                    out=od[:], in0=b2[:], scalar1=4, scalar2=None,
                    op0=mybir.AluOpType.logical_shift_right)
                nc.vector.tensor_scalar(
                    out=b3[:], in0=b3[:], scalar1=4, scalar2=None,
                    op0=mybir.AluOpType.logical_shift_left)
                nc.vector.tensor_scalar(
                    out=b4[:], in0=b4[:], scalar1=12, scalar2=None,
                    op0=mybir.AluOpType.logical_shift_left)
                nc.vector.tensor_tensor(out=od[:], in0=od[:], in1=b3[:],
                                        op=mybir.AluOpType.bitwise_or)
                nc.vector.tensor_tensor(out=od[:], in0=od[:], in1=b4[:],
                                        op=mybir.AluOpType.bitwise_or)
                nc.vector.tensor_copy(out=bi_sb[:, 0::2], in_=ev[:])
                nc.vector.tensor_copy(out=bi_sb[:, 1::2], in_=od[:])

            # AllGather node table shard -> full [N, D] int8 table
            nf_in = dramp.tile([NFROWS, NFW], I8)
            nf_all = dramp.tile([N, D], I8)
            nc.gpsimd.dma_start(out=nf_in[:], in_=blob[0:NFROWS, NOFF : NOFF + NFW])
            nc.gpsimd.collective_compute(
                "AllGather", mybir.AluOpType.bypass,
                replica_groups=[list(range(NCORES))],
                ins=[nf_in.opt()], outs=[nf_all.opt()],
            )
            # AllGather ef-block dictionary shard -> full [U, BS] int8 table
            dc_in = dramp.tile([128, DSHW], I8)
            dc_all = dramp.tile([U, BS], I8)
            nc.gpsimd.dma_start(out=dc_in[:], in_=blob[:, DOFF : DOFF + DSHW])
            nc.gpsimd.collective_compute(
                "AllGather", mybir.AluOpType.bypass,
                replica_groups=[list(range(NCORES))],
                ins=[dc_in.opt()], outs=[dc_all.opt()],
            )

            # grouped pre-build: per chunk, gather x_j rows (1) and ef blocks
            # (BPE); cast both to f16; build one-hots. Then per-chunk matmuls
            # accumulate transposed: pa[f,n] += Xg.T @ P ; pb[f,n] += EF.T @ P
            GS = 16
            pgroups = {}

            def group_of(c):
                g = c // GS
                if g not in pgroups:
                    gs = min(GS, CT - g * GS)
                    xgB = xg8p.tile([128, GS * D], I8, tag="xg8")
                    xfB = xf8p.tile([128, GS * D], I8, tag="xf8")
                    for j in range(gs):
                        cc = g * GS + j
                        nc.gpsimd.indirect_dma_start(
                            out=xgB[:, j * D : (j + 1) * D],
                            out_offset=None,
                            in_=nf_all[:],
                            in_offset=bass.IndirectOffsetOnAxis(
                                ap=si_sb[:, cc : cc + 1], axis=0),
                        )
                        for k in range(BPE):
                            nc.gpsimd.indirect_dma_start(
                                out=xfB[:, j * D + k * BS : j * D + (k + 1) * BS],
                                out_offset=None,
                                in_=dc_all[:],
                                in_offset=bass.IndirectOffsetOnAxis(
                                    ap=bi_sb[:, cc * BPE + k : cc * BPE + k + 1],
                                    axis=0),
                            )
                    xgB16 = xgp.tile([128, GS * D], F16, tag="xg")
                    nc.vector.tensor_copy(out=xgB16[:, : gs * D],
                                          in_=xgB[:, : gs * D])
                    xfB16 = xfp.tile([128, GS * D], F16, tag="xf")
                    nc.vector.tensor_copy(out=xfB16[:, : gs * D],
                                          in_=xfB[:, : gs * D])
                    PB = sb.tile([128, GS, 128], F16, tag="onehot")
                    nc.vector.tensor_tensor(
                        out=PB[:, :gs, :],
                        in0=dr_sb[:, g * GS : g * GS + gs].to_broadcast(
                            [128, gs, 128]),
                        in1=iotaB[:, :gs, :],
                        op=mybir.AluOpType.is_equal,
                    )
                    pgroups[g] = (xgB16, xfB16, PB)
                return pgroups[g], c - g * GS

            obig = constp.tile([128, SLOTS, D], F32)
            cur = 0
            for s in range(SLOTS):
                pa = psa.tile([D, 128], F32, tag="pa")
                pb = psb.tile([D, 128], F32, tag="pb")
                nch = int(cc_counts[s])
                for j in range(nch):
                    c = cur + j
                    (xgB16, xfB16, PB), gj = group_of(c)
                    nc.tensor.matmul(
                        out=pa[:], lhsT=xgB16[:, gj * D : (gj + 1) * D],
                        rhs=PB[:, gj, :],
                        start=(j == 0), stop=(j == nch - 1),
                    )
                    nc.tensor.matmul(
                        out=pb[:], lhsT=xfB16[:, gj * D : (gj + 1) * D],
                        rhs=PB[:, gj, :],
                        start=(j == 0), stop=(j == nch - 1),
                    )
                cur += nch

                aT = ep.tile([D, 128], F16, tag="aT")
                nc.vector.tensor_copy(out=aT[:], in_=pa[:])
                bT = ep.tile([D, 128], F16, tag="bT")
                nc.vector.tensor_copy(out=bT[:], in_=pb[:])
                ops = pso.tile([128, D], F32, tag="ops")
                nc.tensor.matmul(out=ops[:], lhsT=aT[:], rhs=wt_sb, start=True, stop=False)
                nc.tensor.matmul(out=ops[:], lhsT=bT[:], rhs=wb_sb, start=False, stop=True)
                nc.vector.tensor_copy(out=obig[:, s, :], in_=ops[:])

            # batched quantization epilogue over all slots
            am = constp.tile([128, SLOTS], F32)
            nc.vector.tensor_reduce(
                out=am[:], in_=obig[:, :, :], axis=mybir.AxisListType.X,
                op=mybir.AluOpType.max, apply_absolute_value=True)
            nc.vector.tensor_scalar_max(out=am[:], in0=am[:], scalar1=1e-12)
            rc = constp.tile([128, SLOTS], F32)
            nc.vector.reciprocal(out=rc[:], in_=am[:])
            nc.vector.tensor_scalar_mul(out=rc[:], in0=rc[:], scalar1=127.0)
            osb = constp.tile([128, SLOTS, D + 4], I8)
            nc.vector.tensor_tensor(
                out=osb[:, :, 0:D], in0=obig[:, :, :],
                in1=rc[:, :].to_broadcast([128, SLOTS, D]),
                op=mybir.AluOpType.mult)
            nc.vector.tensor_scalar_mul(
                out=osb[:, :, D : D + 4].bitcast(F32), in0=am[:, :],
                scalar1=1.0 / 127.0)
            nc.sync.dma_start(out=outm[:, :], in_=osb[:])
    nc.compile()
    return nc


def _prep(node_feature, edge_feature, edge_index, edge_mask):
    """Host shard: returns (cc_counts, U, blobs, scales, tiles_of_core)."""
    src = np.asarray(edge_index[0], dtype=np.int64)
    dst = np.asarray(edge_index[1], dtype=np.int64)
    keep = np.asarray(edge_mask, dtype=bool)
    src, dst = src[keep], dst[keep]
    ef32 = np.asarray(edge_feature, dtype=np.float32)[keep]
    ef_scale = float(np.abs(ef32).max()) / 127.0
    ef = np.clip(np.round(ef32 / ef_scale), -127, 127).astype(np.int8)
    nf32 = np.asarray(node_feature, dtype=np.float32)
    nf_scale = float(np.abs(nf32).max()) / 127.0
    nf8 = np.clip(np.round(nf32 / nf_scale), -127, 127).astype(np.int8)

    # dictionary-code the int8 ef stream as aligned 8-byte blocks
    blocks = ef.reshape(-1, BS)
    bv = blocks.view([('', np.int8)] * BS).ravel()
    uniq, inv = np.unique(bv, return_inverse=True)
    nuniq = len(uniq)
    U = ((nuniq + NCORES * 128 - 1) // (NCORES * 128)) * (NCORES * 128)
    dict_full = np.zeros((U, BS), np.int8)
    dict_full[:nuniq] = uniq.view(np.int8).reshape(nuniq, BS)
    eidx = inv.astype(np.int32).reshape(-1, BPE)   # [kept_E, BPE]

    tid = dst >> 7
    order = np.argsort(tid, kind="stable")
    src, dst, eidx, tid = src[order], dst[order], eidx[order], tid[order]
    cnt = np.bincount(tid, minlength=NTP)
    starts = np.concatenate([[0], np.cumsum(cnt)])

    # snake-deal tiles (desc count) to cores
    rank = np.argsort(-cnt, kind="stable")
    tiles_of_core = [[] for _ in range(NCORES)]
    for r, t in enumerate(rank):
        blk, pos = divmod(r, NCORES)
        c = pos if blk % 2 == 0 else NCORES - 1 - pos
        tiles_of_core[c].append(int(t))

    # per-slot chunk counts: max over cores
    cc_counts = np.ones(SLOTS, np.int64)
    for s in range(SLOTS):
        m = max(cnt[tiles_of_core[c][s]] for c in range(NCORES))
        cc_counts[s] = max(1, math.ceil(m / PT))
    CT = int(cc_counts.sum())
    DSH = U // NCORES
    DSHW = DSH // 128 * BS
    ix_w, mt_w, w_w, bpp = _offsets(CT, DSHW)
    MOFF = ix_w
    WOFF = ix_w + mt_w
    NOFF = WOFF + w_w
    DOFF = NOFF + NFW

    blobs = []
    for c in range(NCORES):
        ia = np.zeros((CT * PT, BPE), np.int32)
        mt = np.full(CT * PT, 255, np.int32)  # pad: src=0, dst_rel=255
        cur = 0
        for s in range(SLOTS):
            t = tiles_of_core[c][s]
            e0, e1 = starts[t], starts[t] + cnt[t]
            n = e1 - e0
            o = cur * PT
            ia[o : o + n] = eidx[e0:e1]
            mt[o : o + n] = (src[e0:e1] * 256 + (dst[e0:e1] - t * PT)).astype(np.int32)
            cur += int(cc_counts[s])
        blob = np.zeros((PT, bpp), np.int8)
        # idx region: partition-major [chunk, slot(p), BPE] -> rows p,
        # int24 little-endian
        iam = ia.reshape(CT, PT, BPE).transpose(1, 0, 2).reshape(PT, CT * BPE)
        ev, od = iam[:, 0::2], iam[:, 1::2]
        i20 = np.empty((PT, CT * BPE // 2, 5), np.uint8)
        i20[:, :, 0] = ev & 255
        i20[:, :, 1] = (ev >> 8) & 255
        i20[:, :, 2] = ((ev >> 16) & 15) | ((od & 15) << 4)
        i20[:, :, 3] = (od >> 4) & 255
        i20[:, :, 4] = (od >> 12) & 255
        blob[:, :ix_w] = i20.reshape(PT, ix_w).view(np.int8)
        mtp = np.ascontiguousarray(mt.reshape(CT, PT).T)  # [128, CT] int32
        m24 = np.empty((PT, CT, 3), np.uint8)
        m24[:, :, 0] = mtp & 255           # dst_rel (pad rows carry 255)
        m24[:, :, 1] = (mtp >> 8) & 255    # src low
        m24[:, :, 2] = (mtp >> 16) & 255   # src high
        blob[:, MOFF : MOFF + mt_w] = m24.reshape(PT, mt_w).view(np.int8)
        nfseg = np.ascontiguousarray(
            nf8[c * NSH:(c + 1) * NSH]).reshape(NFROWS, NFW)
        blob[0:NFROWS, NOFF : NOFF + NFW] = nfseg
        blob[:, DOFF : DOFF + DSHW] = (
            dict_full[c * DSH:(c + 1) * DSH].reshape(PT, DSHW))
        blobs.append(blob)
    return cc_counts, U, blobs, (ef_scale, nf_scale), tiles_of_core


def _in_maps(inputs, cc_counts, U, blobs, scales):
    ef_scale, nf_scale = scales
    CT = int(sum(cc_counts))
    DSHW = U // NCORES // 128 * BS
    ix_w, mt_w, w_w, bpp = _offsets(CT, DSHW)
    WOFF = ix_w + mt_w
    w32 = np.asarray(inputs["W_msg"], np.float32)
    consts = np.zeros((128, 192), np.float16)
    consts[:96, 0:96] = (w32[:96] * nf_scale).astype(np.float16)
    consts[:96, 96:192] = (w32[96:] * ef_scale).astype(np.float16)
    cbytes = consts.view(np.int8)
    maps = []
    for c in range(NCORES):
        b = blobs[c].copy()
        b[:, WOFF : WOFF + w_w] = cbytes
        maps.append({"blob": b})
    return maps


def make_in_maps(inputs, prep):
    return _in_maps(inputs, prep[0], prep[1], prep[2], prep[3])


def kernel(node_feature, edge_feature, edge_index, edge_mask, W_msg):
    from concourse.bass_utils import run_bass_kernel_spmd

    cc_counts, U, blobs, scales, tiles_of_core = _prep(
        node_feature, edge_feature, edge_index, edge_mask)
    nc = _build(cc_counts, U)
    in_maps = _in_maps({"W_msg": W_msg}, cc_counts, U, blobs, scales)

    res = run_bass_kernel_spmd(nc, in_maps, list(range(NCORES)))

    out_full = np.zeros((NTP * PT, D), np.float32)
    for c in range(NCORES):
        om = res.results[c]["outm"].reshape(128, SLOTS, D + 4)
        q = om[:, :, :D].astype(np.float32)          # [128, SLOTS, D]
        scl = np.ascontiguousarray(om[:, :, D:]).view(np.float32)[:, :, 0]
        oc = (q * scl[:, :, None]).transpose(1, 0, 2)  # [SLOTS, 128, D]
        for s in range(SLOTS):
            t = tiles_of_core[c][s]
            out_full[t * PT : (t + 1) * PT] = oc[s]
    return out_full[:N]


def build_from_prep(prep):
    return _build(prep[0], prep[1])
